# revision 20
# baseline (speedup 1.0000x reference)
"""GQA causal self-attention on 8 Trainium2 NeuronCores.

Problem: B=2, T=2048, C=2048, H=16 query heads, HKV=4 kv heads, HD=128.
Sharding: core (b, g) for b in {0,1}, g in {0..3} owns batch b, kv head g,
and the 4 query heads h with h % 4 == g (reference's _expand_kv maps query
head h -> kv head h % HKV).  Each core computes its heads' attention output
and a partial output projection (its 512 rows of Wp); the host sums the 4
partials per batch and adds bp.  No cross-core communication on device.

All DRAM inputs are host-repacked to [128 partitions, ...contiguous]
layouts so every DMA chunk is >=4KB contiguous per partition (fast issue,
fast transfer).

Device math per core (all matmuls fp16 operands, fp32 PSUM accumulation):
  qT[d, t] = Wq_g.T @ x_b.T      (x is fed pre-transposed from host)
  kT[d, t] = Wk_g.T @ x_b.T
  v[t, d]  = x_b @ Wv_g          (lhsT = xT tiles)
  ST[j, i] = kT_j . qT_i         (j keys on partitions, i queries free)
  A = exp(ST / sqrt(HD)); causal: off-diagonal key tiles computed full
      width, the 4 diagonal key tiles of each i-tile computed only on
      their live query ranges (512/384/256/128 wide) with a tril mask on
      the single triangular 128-block each
  den[*, i] = ones-matmul over gpsimd-accumulated row sums
  yT[d, i] = (sum_j v[j, d] A[j, i]) / den[i]
  out[i, o] += yT.T @ Wp_g       (partial, fp16; host sums over g)
"""

import math
import os
from contextlib import ExitStack

import numpy as np

import concourse.bass as bass
import concourse.mybir as mybir
import concourse.tile as tile
from concourse import bacc, bass_utils

# The axon trace path needs antenv.axon_hooks; if the environment requests
# tracing but lacks the hook module, force tracing off instead of crashing.
if os.environ.get("BASS_TRACE"):
    try:
        import antenv.axon_hooks  # noqa: F401
    except ImportError:
        os.environ["BASS_NEVER_TRACE"] = "1"

# Problem shapes (hardcoded per contest rules).
B, T, C = 2, 2048, 2048
H, G = 16, 4
HKV = H // G          # 4 kv heads
HD = C // H           # 128 head dim
P = 128               # partitions
NH = H // HKV         # 4 local query heads per core
KT = C // P           # 16 contraction tiles for projections
TW = 512              # token tile width (matmul free dim)
NT = T // TW          # 4 token tiles
JTN = T // P          # 16 key tiles of 128
SCALE = 1.0 / math.sqrt(HD)

FP = mybir.dt.float16
F32 = mybir.dt.float32

_CACHE = {}

# Set by kernel() after each run: bass_utils.BassKernelResults.
LAST_RESULT = None


def _build_bass():
    nc = bacc.Bacc("TRN2")

    # Host-packed layouts: partition dim first, then contiguous payload.
    xt = nc.dram_tensor("xt", [P, NT, KT, TW], FP, kind="ExternalInput")
    wq = nc.dram_tensor("wq", [P, KT, NH * HD], FP, kind="ExternalInput")
    wk = nc.dram_tensor("wk", [P, KT, HD], FP, kind="ExternalInput")
    wv = nc.dram_tensor("wv", [P, KT, HD], FP, kind="ExternalInput")
    wp = nc.dram_tensor("wp", [P, NH, C], FP, kind="ExternalInput")
    bq = nc.dram_tensor("bq", [P, NH], F32, kind="ExternalInput")
    bk = nc.dram_tensor("bk", [P, 1], F32, kind="ExternalInput")
    bv = nc.dram_tensor("bv", [HD], F32, kind="ExternalInput")
    mask = nc.dram_tensor("mask", [P, 2, P], FP, kind="ExternalInput")
    out = nc.dram_tensor("out", [T, C], FP, kind="ExternalOutput")

    out_r = out.ap().rearrange("(io p) o -> p io o", p=P)     # [128,16,2048]

    with tile.TileContext(nc) as tc, ExitStack() as ctx:
        consts = ctx.enter_context(tc.tile_pool(name="consts", bufs=1))
        xpool = ctx.enter_context(tc.tile_pool(name="xpool", bufs=2))
        espool = ctx.enter_context(tc.tile_pool(name="espool", bufs=12))
        mpool = ctx.enter_context(tc.tile_pool(name="mpool", bufs=6))
        opool = ctx.enter_context(tc.tile_pool(name="opool", bufs=3))
        # PSUM (8 banks): ps_s 2x[128,2,512] (4) for S pairs + q proj,
        # ps_y 1x[128,512] (1) k-proj + AV, ps_d 1x[128,512] (1) l2-S + den,
        # ps_o 2x[128,512] (2) v-proj + out-proj quarters + warmup.
        ps_s = ctx.enter_context(tc.tile_pool(name="ps_s", bufs=2, space="PSUM"))
        ps_y = ctx.enter_context(tc.tile_pool(name="ps_y", bufs=1, space="PSUM"))
        ps_d = ctx.enter_context(tc.tile_pool(name="ps_d", bufs=1, space="PSUM"))
        ps_o = ctx.enter_context(tc.tile_pool(name="ps_o", bufs=2, space="PSUM"))

        KC = 4  # k-chunks per load
        wq_sb = consts.tile([P, KT, NH * HD], FP)
        wk_sb = consts.tile([P, KT, HD], FP)
        wv_sb = consts.tile([P, KT, HD], FP)
        mask_sb = consts.tile([P, 2, P], FP)
        xtile0 = xpool.tile([P, KT, TW], FP, tag="xt", name="xtile0")
        # Stripe the startup set across the sync and scalar rings in
        # consumption order: each ring runs ~150-250 GB/s while the PE
        # wants ~230 GB/s during the first projection tile.
        for c4 in range(KC):
            ks = slice(c4 * (KT // KC), (c4 + 1) * (KT // KC))
            nc.sync.dma_start(out=wq_sb[:, ks], in_=wq.ap()[:, ks])
            nc.scalar.dma_start(out=xtile0[:, ks], in_=xt.ap()[:, 0, ks])
            nc.sync.dma_start(out=wk_sb[:, ks], in_=wk.ap()[:, ks])
            nc.scalar.dma_start(out=wv_sb[:, ks], in_=wv.ap()[:, ks])
        nc.scalar.dma_start(out=mask_sb, in_=mask.ap())
        bq_sb = consts.tile([P, NH], F32)
        nc.scalar.dma_start(out=bq_sb, in_=bq.ap())
        bk_sb = consts.tile([P, 1], F32)
        nc.scalar.dma_start(out=bk_sb, in_=bk.ap())
        # bv broadcast across partitions (DRAM source allows partition step 0).
        bv_bc = consts.tile([P, HD], F32)
        bv_ap = bass.AP(tensor=bv.ap().tensor, offset=0, ap=[[0, P], [1, HD]])
        nc.scalar.dma_start(out=bv_bc, in_=bv_ap)
        ones_sb = consts.tile([P, P], FP)
        nc.vector.memset(ones_sb, 1.0)
        dummy_sb = consts.tile([P, TW], FP)
        nc.vector.memset(dummy_sb, 0.0)

        # PE warm-up: the first real matmul is gated on the startup DMAs
        # until ~12.5us, so run back-to-back throwaway matmuls in that
        # otherwise-idle window.  HAM un-throttles (1.2 -> 2.4 GHz) after
        # ~3.4us of sustained activity, so the real stream starts warm.
        wa = ps_o.tile([P, TW], F32, tag="pso", name="ps_warm_a")
        wb = ps_o.tile([P, TW], F32, tag="pso", name="ps_warm_b")
        for w in range(12):
            nc.tensor.matmul(
                wa if w % 2 == 0 else wb,
                lhsT=ones_sb,
                rhs=dummy_sb,
                start=True,
                stop=True,
            )

        # Persistent activations.
        qT = consts.tile([P, NH, T], FP)       # [d, h, i]
        kT = consts.tile([P, T], FP)           # [d, j]
        v_sb = consts.tile([P, JTN, HD], FP)   # [j_in, j_tile, d]
        yT = consts.tile([P, NH, T], FP)       # [d, h, i]

        def emit_diag_S(it, h, acc):
            """S + exp + mask + row-sum for the 4 diagonal key tiles of
            (it, h), live query ranges only.  diagA pair: u0 <- l0 (full
            512); u1 <- l1 (384-wide, queries [128:512)) at [0:384) ++ l3
            (128-wide, queries [384:512)) at [384:512).  l2 (256-wide,
            queries [256:512)) rides the ps_d bank.  Returns (esd, es2)."""
            isl = slice(it * TW, (it + 1) * TW)
            jd = 4 * it
            diagA = ps_s.tile([P, 2, TW], F32, tag="pss", name=f"dgA_{it}_{h}")
            psd_s = ps_d.tile([P, TW], F32, tag="psd", name=f"psl2_{it}_{h}")
            nc.tensor.matmul(
                diagA[:, 0, :],
                lhsT=kT[:, jd * P:(jd + 1) * P],
                rhs=qT[:, h, isl],
                start=True, stop=True,
            )
            nc.tensor.matmul(
                diagA[:, 1, 0:384],
                lhsT=kT[:, (jd + 1) * P:(jd + 2) * P],
                rhs=qT[:, h, it * TW + P:(it + 1) * TW],
                start=True, stop=True,
            )
            nc.tensor.matmul(
                diagA[:, 1, 384:512],
                lhsT=kT[:, (jd + 3) * P:(jd + 4) * P],
                rhs=qT[:, h, it * TW + 3 * P:(it + 1) * TW],
                start=True, stop=True,
            )
            nc.tensor.matmul(
                psd_s[:, 0:256],
                lhsT=kT[:, (jd + 2) * P:(jd + 3) * P],
                rhs=qT[:, h, it * TW + 2 * P:(it + 1) * TW],
                start=True, stop=True,
            )
            esd = espool.tile([P, 2, TW], FP, tag="es", name=f"esd_{it}_{h}")
            nc.scalar.activation(
                out=esd, in_=diagA,
                func=mybir.ActivationFunctionType.Exp, scale=SCALE,
            )
            es2 = espool.tile([P, 2, TW], FP, tag="es", name=f"es2_{it}_{h}")
            nc.scalar.activation(
                out=es2[:, 0, 0:256], in_=psd_s[:, 0:256],
                func=mybir.ActivationFunctionType.Exp, scale=SCALE,
            )
            # tril masks on the triangular 128-blocks
            nc.vector.tensor_mul(esd[:, :, 0:P], esd[:, :, 0:P], mask_sb)
            nc.vector.tensor_mul(
                esd[:, 1, 384:512], esd[:, 1, 384:512], mask_sb[:, 0, :]
            )
            nc.vector.tensor_mul(
                es2[:, 0, 0:P], es2[:, 0, 0:P], mask_sb[:, 0, :]
            )
            # row sums into the pair acc
            if it == 0:
                nc.vector.tensor_copy(out=acc[:, 0, :], in_=esd[:, 0, :])
                nc.vector.memset(acc[:, 1, 0:P], 0.0)
                nc.vector.tensor_copy(
                    out=acc[:, 1, P:TW], in_=esd[:, 1, 0:384]
                )
            else:
                nc.vector.tensor_tensor(
                    out=acc[:, 0, :], in0=acc[:, 0, :],
                    in1=esd[:, 0, :], op=mybir.AluOpType.add,
                )
                nc.vector.tensor_tensor(
                    out=acc[:, 1, P:TW], in0=acc[:, 1, P:TW],
                    in1=esd[:, 1, 0:384], op=mybir.AluOpType.add,
                )
            nc.vector.tensor_tensor(
                out=acc[:, 0, 3 * P:TW], in0=acc[:, 0, 3 * P:TW],
                in1=esd[:, 1, 384:512], op=mybir.AluOpType.add,
            )
            nc.vector.tensor_tensor(
                out=acc[:, 1, 2 * P:TW], in0=acc[:, 1, 2 * P:TW],
                in1=es2[:, 0, 0:256], op=mybir.AluOpType.add,
            )
            return esd, es2

        def emit_diag_AV(it, h, psy, esd, es2):
            jd = 4 * it
            if it == 0:
                # l0 first (start=True full width), l2 carries stop.
                nc.tensor.matmul(
                    psy, lhsT=v_sb[:, jd, :], rhs=esd[:, 0, :],
                    start=True, stop=False,
                )
            nc.tensor.matmul(
                psy[:, P:TW], lhsT=v_sb[:, jd + 1, :],
                rhs=esd[:, 1, 0:384], start=False, stop=False,
            )
            nc.tensor.matmul(
                psy[:, 3 * P:TW], lhsT=v_sb[:, jd + 3, :],
                rhs=esd[:, 1, 384:512], start=False, stop=False,
            )
            nc.tensor.matmul(
                psy[:, 2 * P:TW], lhsT=v_sb[:, jd + 2, :],
                rhs=es2[:, 0, 0:256], start=False, stop=(it == 0),
            )
            if it > 0:
                nc.tensor.matmul(
                    psy, lhsT=v_sb[:, jd, :], rhs=esd[:, 0, :],
                    start=False, stop=True,
                )

        def emit_den_yT(it, h, psy, acc):
            accs = mpool.tile([P, TW], FP, tag="accs", name=f"accs_{it}_{h}")
            nc.vector.tensor_tensor(
                out=accs, in0=acc[:, 0, :], in1=acc[:, 1, :],
                op=mybir.AluOpType.add,
            )
            psd = ps_d.tile([P, TW], F32, tag="psd", name=f"psd_{it}_{h}")
            nc.tensor.matmul(
                psd, lhsT=ones_sb, rhs=accs, start=True, stop=True
            )
            return psd

        def finish_yT(it, h, psy, psd):
            rb = mpool.tile([P, TW], F32, tag="rb")
            nc.vector.reciprocal_approx_fast(out=rb, in_=psd)
            nc.vector.tensor_mul(
                yT[:, h, it * TW:(it + 1) * TW], psy, rb
            )

        # it=0 attention state, produced inside the projection phase (ACT
        # and DVE are idle there) and consumed right after it.
        acc0 = {}
        es0 = {}

        # ---- Projections ----
        # Stream the contraction dim: per 4-k chunk, feed all accumulators
        # (4 q heads, k, 4 v token blocks) so compute starts as soon as the
        # first chunk of wq/xt lands instead of after the full 4MB.
        for n in range(NT):
            if n == 0:
                xtile = xtile0
            else:
                xtile = xpool.tile([P, KT, TW], FP, tag="xt", name=f"xtile{n}")
                for c4 in range(KC):
                    ks = slice(c4 * (KT // KC), (c4 + 1) * (KT // KC))
                    eng = nc.sync if c4 % 2 == 0 else nc.scalar
                    eng.dma_start(out=xtile[:, ks], in_=xt.ap()[:, n, ks])
            psq01 = ps_s.tile([P, 2, TW], F32, tag="pss", name=f"psq01_{n}")
            psq23 = ps_s.tile([P, 2, TW], F32, tag="pss", name=f"psq23_{n}")
            psk = ps_y.tile([P, TW], F32, tag="psy", name=f"psk_{n}")
            for k in range(KT):
                st = k == 0
                sp = k == KT - 1
                for h in range(NH):
                    tgt = psq01 if h < 2 else psq23
                    nc.tensor.matmul(
                        tgt[:, h % 2, :],
                        lhsT=wq_sb[:, k, h * HD:(h + 1) * HD],
                        rhs=xtile[:, k, :],
                        start=st,
                        stop=sp,
                    )
                nc.tensor.matmul(
                    psk, lhsT=wk_sb[:, k, :], rhs=xtile[:, k, :], start=st, stop=sp
                )
            for h in range(NH):
                tgt = psq01 if h < 2 else psq23
                nc.vector.tensor_scalar(
                    out=qT[:, h, n * TW:(n + 1) * TW],
                    in0=tgt[:, h % 2, :],
                    scalar1=bq_sb[:, h:h + 1],
                    scalar2=None,
                    op0=mybir.AluOpType.add,
                )
            nc.vector.tensor_scalar(
                out=kT[:, n * TW:(n + 1) * TW],
                in0=psk,
                scalar1=bk_sb,
                scalar2=None,
                op0=mybir.AluOpType.add,
            )
            # v-projection: DMA-independent by now (q/k streamed the whole
            # xtile); single-bank accumulators.
            for js in range(TW // P):
                psv = ps_o.tile([P, TW], F32, tag="pso", name=f"psv_{n}_{js}")
                for k in range(KT):
                    nc.tensor.matmul(
                        psv[:, :HD],
                        lhsT=xtile[:, k, js * P:(js + 1) * P],
                        rhs=wv_sb[:, k, :],
                        start=(k == 0),
                        stop=(k == KT - 1),
                    )
                nc.vector.tensor_tensor(
                    out=v_sb[:, n * (TW // P) + js, :],
                    in0=psv[:, :HD],
                    in1=bv_bc,
                    op=mybir.AluOpType.add,
                )
            # Hoist it=0's S/exp/mask/row-sum work between projection
            # tiles: the PE is projection-bound here while ACT/DVE idle,
            # and it=0's exps otherwise backlog ACT into i-tile 1.
            if n >= 1:
                for h in (0, 1) if n == 1 else (2, 3) if n == 2 else ():
                    acc = mpool.tile(
                        [P, 2, TW], FP, tag="acc", name=f"acc_0_{h}"
                    )
                    es0[h] = emit_diag_S(0, h, acc)
                    acc0[h] = acc

        # Weights for the out-projection: load after projection work is
        # queued (scalar ring; it is idle by then).
        wp_sb = consts.tile([P, NH, C], FP)
        nc.scalar.dma_start(out=wp_sb, in_=wp.ap())

        # ---- Attention with interleaved output projection ----
        def out_proj_quarter(ic, ot):
            # 4 accumulating matmuls for one 512-col quarter of chunk ic
            # in a single PSUM bank.
            pso = ps_o.tile([P, TW], F32, tag="pso", name=f"pso_{ic}_{ot}")
            for h in range(NH):
                nc.tensor.matmul(
                    pso,
                    lhsT=yT[:, h, ic * P:(ic + 1) * P],
                    rhs=wp_sb[:, h, ot * TW:(ot + 1) * TW],
                    start=(h == 0),
                    stop=(h == NH - 1),
                )
            return pso

        def out_proj_copy(ic, ot, osb, pso):
            # alternate the psum drain between scalar and vector
            if ot % 2 == 0:
                nc.scalar.activation(
                    out=osb[:, ot * TW:(ot + 1) * TW],
                    in_=pso,
                    func=mybir.ActivationFunctionType.Copy,
                )
            else:
                nc.vector.tensor_copy(
                    out=osb[:, ot * TW:(ot + 1) * TW], in_=pso
                )
            if ot == 3:
                nc.sync.dma_start(out=out_r[:, ic, :], in_=osb)

        # i-tile 0: S/exp already done during projections; just AV + den.
        for h in range(NH):
            psy = ps_y.tile([P, TW], F32, tag="psy", name=f"psy_0_{h}")
            esd, es2 = es0[h]
            emit_diag_AV(0, h, psy, esd, es2)
            psd = emit_den_yT(0, h, psy, acc0[h])
            finish_yT(0, h, psy, psd)

        for it in range(1, NT):
            isl = slice(it * TW, (it + 1) * TW)
            noff = 2 * it          # full-width off-diagonal key-tile pairs
            for h in range(NH):
                ic = (it - 1) * (TW // P) + h   # out-proj chunk to interleave
                osb = opool.tile([P, C], FP, tag="osb", name=f"osb_{ic}")
                psy = ps_y.tile([P, TW], F32, tag="psy", name=f"psy_{it}_{h}")
                acc = mpool.tile([P, 2, TW], FP, tag="acc", name=f"acc_{it}_{h}")
                es_list = {}

                def emit_S(pr):
                    jt0 = 2 * pr
                    pss = ps_s.tile(
                        [P, 2, TW], F32, tag="pss", name=f"pss_{it}_{h}_{pr}"
                    )
                    for u in range(2):
                        nc.tensor.matmul(
                            pss[:, u, :],
                            lhsT=kT[:, (jt0 + u) * P:(jt0 + u + 1) * P],
                            rhs=qT[:, h, isl],
                            start=True,
                            stop=True,
                        )
                    es = espool.tile([P, 2, TW], FP, tag="es")
                    nc.scalar.activation(
                        out=es,
                        in_=pss,
                        func=mybir.ActivationFunctionType.Exp,
                        scale=SCALE,
                    )
                    if pr == 0:
                        nc.vector.tensor_copy(out=acc, in_=es)
                    else:
                        nc.vector.tensor_tensor(
                            out=acc, in0=acc, in1=es, op=mybir.AluOpType.add
                        )
                    es_list[pr] = es

                def emit_AV(pr):
                    jt0 = 2 * pr
                    es = es_list[pr]
                    for u in range(2):
                        nc.tensor.matmul(
                            psy,
                            lhsT=v_sb[:, jt0 + u, :],
                            rhs=es[:, u, :],
                            start=(jt0 + u == 0),
                            stop=False,
                        )

                # --- emission schedule: keep the PE 2 psum slots ahead ---
                emit_S(0)
                emit_S(1)
                psoA0 = out_proj_quarter(ic, 0)
                dias = None
                for p in range(noff):
                    emit_AV(p)
                    nxt = p + 2
                    if nxt < noff:
                        emit_S(nxt)
                    elif nxt == noff:
                        dias = emit_diag_S(it, h, acc)
                psoA1 = out_proj_quarter(ic, 1)
                emit_diag_AV(it, h, psy, dias[0], dias[1])
                psd = emit_den_yT(it, h, psy, acc)
                out_proj_copy(ic, 0, osb, psoA0)
                out_proj_copy(ic, 1, osb, psoA1)
                psoB2 = out_proj_quarter(ic, 2)
                out_proj_copy(ic, 2, osb, psoB2)
                psoB3 = out_proj_quarter(ic, 3)
                out_proj_copy(ic, 3, osb, psoB3)
                finish_yT(it, h, psy, psd)
        # Tail chunks: attention is done, so the S-pair pool (2 slots) is
        # also free and gives quarter-to-quarter pipelining.
        for h in range(NH):
            ic = (NT - 1) * (TW // P) + h
            osb = opool.tile([P, C], FP, tag="osb", name=f"osb_{ic}")
            for ot in range(4):
                pso = out_proj_quarter(ic, ot)
                out_proj_copy(ic, ot, osb, pso)

    nc.compile()
    return nc


def _causal_mask_tiles():
    # [128, 2, 128] tril (key_in_tile <= query_in_block), both slots equal.
    j = np.arange(P)[:, None, None]
    i = np.arange(P)[None, None, :]
    return np.broadcast_to(j <= i, (P, 2, P)).astype(np.float16)


def kernel(x, Wkv, bkv, Wq, bq, Wp, bp):
    global LAST_RESULT
    x = np.asarray(x, np.float32)
    Wkv = np.asarray(Wkv, np.float32)
    bkv = np.asarray(bkv, np.float32)
    Wq = np.asarray(Wq, np.float32)
    bq = np.asarray(bq, np.float32)
    Wp = np.asarray(Wp, np.float32)
    bp = np.asarray(bp, np.float32)

    if "nc" not in _CACHE:
        _CACHE["nc"] = _build_bass()
    nc = _CACHE["nc"]

    mask = _causal_mask_tiles()
    CG = C // G  # 512 columns per kv head in the k/v halves of Wkv

    in_maps = []
    for b in range(B):
        # xt packed: [p, n, ko, tw] = x[b].T[ko*128+p, n*512+tw]
        xtb = x[b].T.astype(np.float16).reshape(KT, P, NT, TW)
        xt_packed = np.ascontiguousarray(xtb.transpose(1, 2, 0, 3))
        for g in range(HKV):
            heads = [g + HKV * u for u in range(NH)]  # h % HKV == g
            wq_g = np.concatenate(
                [Wq[:, h * HD:(h + 1) * HD] for h in heads], axis=1
            ).astype(np.float16)
            wq_p = np.ascontiguousarray(
                wq_g.reshape(KT, P, NH * HD).transpose(1, 0, 2)
            )
            bq_g = np.concatenate([bq[h * HD:(h + 1) * HD] for h in heads])
            bq_p = np.ascontiguousarray(
                bq_g.reshape(NH, P).T.astype(np.float32)
            )
            wp_g = np.concatenate(
                [Wp[h * HD:(h + 1) * HD, :] for h in heads], axis=0
            ).astype(np.float16)
            wp_p = np.ascontiguousarray(wp_g.reshape(NH, P, C).transpose(1, 0, 2))
            wk_g = Wkv[:, g * HD:(g + 1) * HD].astype(np.float16)
            wk_p = np.ascontiguousarray(wk_g.reshape(KT, P, HD).transpose(1, 0, 2))
            wv_g = Wkv[:, CG + g * HD:CG + (g + 1) * HD].astype(np.float16)
            wv_p = np.ascontiguousarray(wv_g.reshape(KT, P, HD).transpose(1, 0, 2))
            bk_g = np.ascontiguousarray(
                bkv[g * HD:(g + 1) * HD].reshape(P, 1), np.float32
            )
            bv_g = np.ascontiguousarray(
                bkv[CG + g * HD:CG + (g + 1) * HD], np.float32
            )
            in_maps.append(
                {
                    "xt": xt_packed,
                    "wq": wq_p,
                    "wk": wk_p,
                    "wv": wv_p,
                    "wp": wp_p,
                    "bq": bq_p,
                    "bk": bk_g,
                    "bv": bv_g,
                    "mask": mask,
                }
            )

    res = bass_utils.run_bass_kernel_spmd(nc, in_maps, core_ids=list(range(B * HKV)))
    LAST_RESULT = res

    out = np.zeros((B, T, C), np.float32)
    for b in range(B):
        acc = np.zeros((T, C), np.float32)
        for g in range(HKV):
            acc += res.results[b * HKV + g]["out"]
        out[b] = acc + bp[None, :]
    return out


# revision 21
# speedup vs baseline: 1.1045x; 1.1045x over previous
"""GQA causal self-attention on 8 Trainium2 NeuronCores.

Problem: B=2, T=2048, C=2048, H=16 query heads, HKV=4 kv heads, HD=128.
Sharding: core (b, g) for b in {0,1}, g in {0..3} owns batch b, kv head g,
and the 4 query heads h with h % 4 == g (reference's _expand_kv maps query
head h -> kv head h % HKV).  Each core computes its heads' attention output
and a partial output projection (its 512 rows of Wp); the host sums the 4
partials per batch and adds bp.  No cross-core communication on device.

All DRAM inputs are host-repacked to [128 partitions, ...contiguous]
layouts so every DMA chunk is >=4KB contiguous per partition (fast issue,
fast transfer).

Device math per core (all matmuls fp16 operands, fp32 PSUM accumulation):
  qT[d, t] = Wq_g.T @ x_b.T      (x is fed pre-transposed from host)
  kT[d, t] = Wk_g.T @ x_b.T
  v[t, d]  = x_b @ Wv_g          (lhsT = xT tiles)
  ST[j, i] = kT_j . qT_i         (j keys on partitions, i queries free)
  A = exp(ST / sqrt(HD)); causal: off-diagonal key tiles computed full
      width, the 4 diagonal key tiles of each i-tile computed only on
      their live query ranges (512/384/256/128 wide) with a tril mask on
      the single triangular 128-block each
  den[*, i] = ones-matmul over gpsimd-accumulated row sums
  yT[d, i] = (sum_j v[j, d] A[j, i]) / den[i]
  out[i, o] += yT.T @ Wp_g       (partial, fp16; host sums over g)
"""

import math
import os
from contextlib import ExitStack

import numpy as np

import concourse.bass as bass
import concourse.mybir as mybir
import concourse.tile as tile
from concourse import bacc, bass_utils

# The axon trace path needs antenv.axon_hooks; if the environment requests
# tracing but lacks the hook module, force tracing off instead of crashing.
if os.environ.get("BASS_TRACE"):
    try:
        import antenv.axon_hooks  # noqa: F401
    except ImportError:
        os.environ["BASS_NEVER_TRACE"] = "1"

# Problem shapes (hardcoded per contest rules).
B, T, C = 2, 2048, 2048
H, G = 16, 4
HKV = H // G          # 4 kv heads
HD = C // H           # 128 head dim
P = 128               # partitions
NH = H // HKV         # 4 local query heads per core
KT = C // P           # 16 contraction tiles for projections
TW = 512              # token tile width (matmul free dim)
NT = T // TW          # 4 token tiles
JTN = T // P          # 16 key tiles of 128
SCALE = 1.0 / math.sqrt(HD)

FP = mybir.dt.float16
F32 = mybir.dt.float32

_CACHE = {}

# Set by kernel() after each run: bass_utils.BassKernelResults.
LAST_RESULT = None


def _build_bass():
    nc = bacc.Bacc("TRN2")

    # Host-packed layouts: partition dim first, then contiguous payload.
    xt = nc.dram_tensor("xt", [P, NT, KT, TW], FP, kind="ExternalInput")
    wq = nc.dram_tensor("wq", [P, KT, NH * HD], FP, kind="ExternalInput")
    wk = nc.dram_tensor("wk", [P, KT, HD], FP, kind="ExternalInput")
    wv = nc.dram_tensor("wv", [P, KT, HD], FP, kind="ExternalInput")
    wp = nc.dram_tensor("wp", [P, NH, C], FP, kind="ExternalInput")
    bq = nc.dram_tensor("bq", [P, NH], F32, kind="ExternalInput")
    bk = nc.dram_tensor("bk", [P, 1], F32, kind="ExternalInput")
    bv = nc.dram_tensor("bv", [HD], F32, kind="ExternalInput")
    mask = nc.dram_tensor("mask", [P, 2, P], FP, kind="ExternalInput")
    ident = nc.dram_tensor("ident", [P, P], FP, kind="ExternalInput")
    out = nc.dram_tensor("out", [T, C], FP, kind="ExternalOutput")

    out_r = out.ap().rearrange("(io p) o -> p io o", p=P)     # [128,16,2048]

    with tile.TileContext(nc) as tc, ExitStack() as ctx:
        consts = ctx.enter_context(tc.tile_pool(name="consts", bufs=1))
        xpool = ctx.enter_context(tc.tile_pool(name="xpool", bufs=2))
        espool = ctx.enter_context(tc.tile_pool(name="espool", bufs=12))
        mpool = ctx.enter_context(tc.tile_pool(name="mpool", bufs=6))
        opool = ctx.enter_context(tc.tile_pool(name="opool", bufs=3))
        vtpool = ctx.enter_context(tc.tile_pool(name="vtpool", bufs=2))
        # PSUM (8 banks): ps_s 2x[128,2,512] (4) for S pairs + q proj,
        # ps_y 1x[128,512] (1) k-proj + AV, ps_d 1x[128,512] (1) l2-S + den,
        # ps_o 2x[128,512] (2) v-proj + out-proj quarters + warmup.
        ps_s = ctx.enter_context(tc.tile_pool(name="ps_s", bufs=2, space="PSUM"))
        ps_y = ctx.enter_context(tc.tile_pool(name="ps_y", bufs=1, space="PSUM"))
        ps_d = ctx.enter_context(tc.tile_pool(name="ps_d", bufs=1, space="PSUM"))
        ps_o = ctx.enter_context(tc.tile_pool(name="ps_o", bufs=2, space="PSUM"))

        KC = 4  # k-chunks per load
        wq_sb = consts.tile([P, KT, NH * HD], FP)
        wk_sb = consts.tile([P, KT, HD], FP)
        wv_sb = consts.tile([P, KT, HD], FP)
        mask_sb = consts.tile([P, 2, P], FP)
        xtile0 = xpool.tile([P, KT, TW], FP, tag="xt", name="xtile0")
        # Stripe the startup set across the sync and scalar rings in
        # consumption order: each ring runs ~150-250 GB/s while the PE
        # wants ~230 GB/s during the first projection tile.
        for c4 in range(KC):
            ks = slice(c4 * (KT // KC), (c4 + 1) * (KT // KC))
            nc.sync.dma_start(out=wq_sb[:, ks], in_=wq.ap()[:, ks])
            nc.scalar.dma_start(out=xtile0[:, ks], in_=xt.ap()[:, 0, ks])
            nc.sync.dma_start(out=wk_sb[:, ks], in_=wk.ap()[:, ks])
            nc.scalar.dma_start(out=wv_sb[:, ks], in_=wv.ap()[:, ks])
        nc.scalar.dma_start(out=mask_sb, in_=mask.ap())
        id_sb = consts.tile([P, P], FP)
        nc.scalar.dma_start(out=id_sb, in_=ident.ap())
        bq_sb = consts.tile([P, NH], F32)
        nc.scalar.dma_start(out=bq_sb, in_=bq.ap())
        bk_sb = consts.tile([P, 1], F32)
        nc.scalar.dma_start(out=bk_sb, in_=bk.ap())
        # bv broadcast across partitions (DRAM source allows partition step 0).
        bv_bc = consts.tile([P, HD], F32)
        bv_ap = bass.AP(tensor=bv.ap().tensor, offset=0, ap=[[0, P], [1, HD]])
        nc.scalar.dma_start(out=bv_bc, in_=bv_ap)
        ones_sb = consts.tile([P, P], FP)
        nc.vector.memset(ones_sb, 1.0)
        dummy_sb = consts.tile([P, TW], FP)
        nc.vector.memset(dummy_sb, 0.0)

        # PE warm-up: the first real matmul is gated on the startup DMAs
        # until ~12.5us, so run back-to-back throwaway matmuls in that
        # otherwise-idle window.  HAM un-throttles (1.2 -> 2.4 GHz) after
        # ~3.4us of sustained activity, so the real stream starts warm.
        wa = ps_o.tile([P, TW], F32, tag="pso", name="ps_warm_a")
        wb = ps_o.tile([P, TW], F32, tag="pso", name="ps_warm_b")
        for w in range(12):
            nc.tensor.matmul(
                wa if w % 2 == 0 else wb,
                lhsT=ones_sb,
                rhs=dummy_sb,
                start=True,
                stop=True,
            )

        # Persistent activations.
        qT = consts.tile([P, NH, T], FP)       # [d, h, i]
        kT = consts.tile([P, T], FP)           # [d, j]
        v_sb = consts.tile([P, JTN, HD], FP)   # [j_in, j_tile, d]
        yT = consts.tile([P, NH, T], FP)       # [d, h, i]

        def emit_diag_S(it, h, acc):
            """S + exp + mask + row-sum for the 4 diagonal key tiles of
            (it, h), live query ranges only.  diagA pair: u0 <- l0 (full
            512); u1 <- l1 (384-wide, queries [128:512)) at [0:384) ++ l3
            (128-wide, queries [384:512)) at [384:512).  l2 (256-wide,
            queries [256:512)) rides the ps_d bank.  Returns (esd, es2)."""
            isl = slice(it * TW, (it + 1) * TW)
            jd = 4 * it
            diagA = ps_s.tile([P, 2, TW], F32, tag="pss", name=f"dgA_{it}_{h}")
            psd_s = ps_d.tile([P, TW], F32, tag="psd", name=f"psl2_{it}_{h}")
            nc.tensor.matmul(
                diagA[:, 0, :],
                lhsT=kT[:, jd * P:(jd + 1) * P],
                rhs=qT[:, h, isl],
                start=True, stop=True,
            )
            nc.tensor.matmul(
                diagA[:, 1, 0:384],
                lhsT=kT[:, (jd + 1) * P:(jd + 2) * P],
                rhs=qT[:, h, it * TW + P:(it + 1) * TW],
                start=True, stop=True,
            )
            nc.tensor.matmul(
                diagA[:, 1, 384:512],
                lhsT=kT[:, (jd + 3) * P:(jd + 4) * P],
                rhs=qT[:, h, it * TW + 3 * P:(it + 1) * TW],
                start=True, stop=True,
            )
            nc.tensor.matmul(
                psd_s[:, 0:256],
                lhsT=kT[:, (jd + 2) * P:(jd + 3) * P],
                rhs=qT[:, h, it * TW + 2 * P:(it + 1) * TW],
                start=True, stop=True,
            )
            esd = espool.tile([P, 2, TW], FP, tag="es", name=f"esd_{it}_{h}")
            nc.scalar.activation(
                out=esd, in_=diagA,
                func=mybir.ActivationFunctionType.Exp, scale=SCALE,
            )
            es2 = espool.tile([P, 2, TW], FP, tag="es", name=f"es2_{it}_{h}")
            nc.scalar.activation(
                out=es2[:, 0, 0:256], in_=psd_s[:, 0:256],
                func=mybir.ActivationFunctionType.Exp, scale=SCALE,
            )
            # tril masks on the triangular 128-blocks
            nc.vector.tensor_mul(esd[:, :, 0:P], esd[:, :, 0:P], mask_sb)
            nc.vector.tensor_mul(
                esd[:, 1, 384:512], esd[:, 1, 384:512], mask_sb[:, 0, :]
            )
            nc.vector.tensor_mul(
                es2[:, 0, 0:P], es2[:, 0, 0:P], mask_sb[:, 0, :]
            )
            # row sums into the pair acc
            if it == 0:
                nc.vector.tensor_copy(out=acc[:, 0, :], in_=esd[:, 0, :])
                nc.vector.memset(acc[:, 1, 0:P], 0.0)
                nc.vector.tensor_copy(
                    out=acc[:, 1, P:TW], in_=esd[:, 1, 0:384]
                )
            else:
                nc.vector.tensor_tensor(
                    out=acc[:, 0, :], in0=acc[:, 0, :],
                    in1=esd[:, 0, :], op=mybir.AluOpType.add,
                )
                nc.vector.tensor_tensor(
                    out=acc[:, 1, P:TW], in0=acc[:, 1, P:TW],
                    in1=esd[:, 1, 0:384], op=mybir.AluOpType.add,
                )
            nc.vector.tensor_tensor(
                out=acc[:, 0, 3 * P:TW], in0=acc[:, 0, 3 * P:TW],
                in1=esd[:, 1, 384:512], op=mybir.AluOpType.add,
            )
            nc.vector.tensor_tensor(
                out=acc[:, 1, 2 * P:TW], in0=acc[:, 1, 2 * P:TW],
                in1=es2[:, 0, 0:256], op=mybir.AluOpType.add,
            )
            return esd, es2

        def emit_diag_AV(it, h, psy, esd, es2):
            jd = 4 * it
            if it == 0:
                # l0 first (start=True full width), l2 carries stop.
                nc.tensor.matmul(
                    psy, lhsT=v_sb[:, jd, :], rhs=esd[:, 0, :],
                    start=True, stop=False,
                )
            nc.tensor.matmul(
                psy[:, P:TW], lhsT=v_sb[:, jd + 1, :],
                rhs=esd[:, 1, 0:384], start=False, stop=False,
            )
            nc.tensor.matmul(
                psy[:, 3 * P:TW], lhsT=v_sb[:, jd + 3, :],
                rhs=esd[:, 1, 384:512], start=False, stop=False,
            )
            nc.tensor.matmul(
                psy[:, 2 * P:TW], lhsT=v_sb[:, jd + 2, :],
                rhs=es2[:, 0, 0:256], start=False, stop=(it == 0),
            )
            if it > 0:
                nc.tensor.matmul(
                    psy, lhsT=v_sb[:, jd, :], rhs=esd[:, 0, :],
                    start=False, stop=True,
                )

        def emit_den_yT(it, h, psy, acc):
            accs = mpool.tile([P, TW], FP, tag="accs", name=f"accs_{it}_{h}")
            nc.vector.tensor_tensor(
                out=accs, in0=acc[:, 0, :], in1=acc[:, 1, :],
                op=mybir.AluOpType.add,
            )
            psd = ps_d.tile([P, TW], F32, tag="psd", name=f"psd_{it}_{h}")
            nc.tensor.matmul(
                psd, lhsT=ones_sb, rhs=accs, start=True, stop=True
            )
            return psd

        def finish_yT(it, h, psy, psd):
            rb = mpool.tile([P, TW], F32, tag="rb")
            nc.vector.reciprocal_approx_fast(out=rb, in_=psd)
            nc.vector.tensor_mul(
                yT[:, h, it * TW:(it + 1) * TW], psy, rb
            )

        # it=0 attention state, produced inside the projection phase (ACT
        # and DVE are idle there) and consumed right after it.
        acc0 = {}
        es0 = {}

        vt_state = {"prev": None}

        def emit_v_transpose():
            vtile, np_ = vt_state["prev"]
            pst = ps_o.tile([P, TW], F32, tag="pso", name=f"pst_{np_}")
            for tb in range(TW // P):
                nc.tensor.matmul(
                    pst[:, tb * HD:(tb + 1) * HD],
                    lhsT=vtile[:, tb * P:(tb + 1) * P],
                    rhs=id_sb,
                    start=True,
                    stop=True,
                )
            for tb in range(TW // P):
                nc.vector.tensor_tensor(
                    out=v_sb[:, np_ * (TW // P) + tb, :],
                    in0=pst[:, tb * HD:(tb + 1) * HD],
                    in1=bv_bc,
                    op=mybir.AluOpType.add,
                )

        # ---- Projections ----
        # Stream the contraction dim: per 4-k chunk, feed all accumulators
        # (4 q heads, k, 4 v token blocks) so compute starts as soon as the
        # first chunk of wq/xt lands instead of after the full 4MB.
        for n in range(NT):
            if n == 0:
                xtile = xtile0
            else:
                xtile = xpool.tile([P, KT, TW], FP, tag="xt", name=f"xtile{n}")
                for c4 in range(KC):
                    ks = slice(c4 * (KT // KC), (c4 + 1) * (KT // KC))
                    eng = nc.sync if c4 % 2 == 0 else nc.scalar
                    eng.dma_start(out=xtile[:, ks], in_=xt.ap()[:, n, ks])
            psq01 = ps_s.tile([P, 2, TW], F32, tag="pss", name=f"psq01_{n}")
            psq23 = ps_s.tile([P, 2, TW], F32, tag="pss", name=f"psq23_{n}")
            psk = ps_y.tile([P, TW], F32, tag="psy", name=f"psk_{n}")
            for k in range(KT):
                st = k == 0
                sp = k == KT - 1
                for h in range(NH):
                    tgt = psq01 if h < 2 else psq23
                    nc.tensor.matmul(
                        tgt[:, h % 2, :],
                        lhsT=wq_sb[:, k, h * HD:(h + 1) * HD],
                        rhs=xtile[:, k, :],
                        start=st,
                        stop=sp,
                    )
                nc.tensor.matmul(
                    psk, lhsT=wk_sb[:, k, :], rhs=xtile[:, k, :], start=st, stop=sp
                )
            for h in range(NH):
                tgt = psq01 if h < 2 else psq23
                nc.vector.tensor_scalar(
                    out=qT[:, h, n * TW:(n + 1) * TW],
                    in0=tgt[:, h % 2, :],
                    scalar1=bq_sb[:, h:h + 1],
                    scalar2=None,
                    op0=mybir.AluOpType.add,
                )
            nc.vector.tensor_scalar(
                out=kT[:, n * TW:(n + 1) * TW],
                in0=psk,
                scalar1=bk_sb,
                scalar2=None,
                op0=mybir.AluOpType.add,
            )
            # v-projection: compute v^T with full 512-wide streams (same
            # shape as the k-projection), then transpose 128-blocks on the
            # PE via identity matmuls one n-tile later (so the PE never
            # waits on the intermediate SBUF copy).
            psv2 = ps_o.tile([P, TW], F32, tag="pso", name=f"psv2_{n}")
            for k in range(KT):
                nc.tensor.matmul(
                    psv2,
                    lhsT=wv_sb[:, k, :],
                    rhs=xtile[:, k, :],
                    start=(k == 0),
                    stop=(k == KT - 1),
                )
            if vt_state["prev"] is not None:
                emit_v_transpose()
            vtile = vtpool.tile([P, TW], FP, tag="vt", name=f"vt_{n}")
            nc.vector.tensor_copy(out=vtile, in_=psv2)
            vt_state["prev"] = (vtile, n)
            # Hoist it=0's S/exp/mask/row-sum work between projection
            # tiles: the PE is projection-bound here while ACT/DVE idle,
            # and it=0's exps otherwise backlog ACT into i-tile 1.
            if n >= 1:
                for h in (0, 1) if n == 1 else (2, 3) if n == 2 else ():
                    acc = mpool.tile(
                        [P, 2, TW], FP, tag="acc", name=f"acc_0_{h}"
                    )
                    es0[h] = emit_diag_S(0, h, acc)
                    acc0[h] = acc

        emit_v_transpose()

        # Weights for the out-projection: load after projection work is
        # queued (scalar ring; it is idle by then).
        wp_sb = consts.tile([P, NH, C], FP)
        nc.scalar.dma_start(out=wp_sb, in_=wp.ap())

        # ---- Attention with interleaved output projection ----
        def out_proj_quarter(ic, ot):
            # 4 accumulating matmuls for one 512-col quarter of chunk ic
            # in a single PSUM bank.
            pso = ps_o.tile([P, TW], F32, tag="pso", name=f"pso_{ic}_{ot}")
            for h in range(NH):
                nc.tensor.matmul(
                    pso,
                    lhsT=yT[:, h, ic * P:(ic + 1) * P],
                    rhs=wp_sb[:, h, ot * TW:(ot + 1) * TW],
                    start=(h == 0),
                    stop=(h == NH - 1),
                )
            return pso

        def out_proj_copy(ic, ot, osb, pso):
            # alternate the psum drain between scalar and vector
            if ot % 2 == 0:
                nc.scalar.activation(
                    out=osb[:, ot * TW:(ot + 1) * TW],
                    in_=pso,
                    func=mybir.ActivationFunctionType.Copy,
                )
            else:
                nc.vector.tensor_copy(
                    out=osb[:, ot * TW:(ot + 1) * TW], in_=pso
                )
            if ot == 3:
                nc.sync.dma_start(out=out_r[:, ic, :], in_=osb)

        # i-tile 0: S/exp already done during projections; just AV + den.
        for h in range(NH):
            psy = ps_y.tile([P, TW], F32, tag="psy", name=f"psy_0_{h}")
            esd, es2 = es0[h]
            emit_diag_AV(0, h, psy, esd, es2)
            psd = emit_den_yT(0, h, psy, acc0[h])
            finish_yT(0, h, psy, psd)

        for it in range(1, NT):
            isl = slice(it * TW, (it + 1) * TW)
            noff = 2 * it          # full-width off-diagonal key-tile pairs
            for h in range(NH):
                ic = (it - 1) * (TW // P) + h   # out-proj chunk to interleave
                osb = opool.tile([P, C], FP, tag="osb", name=f"osb_{ic}")
                psy = ps_y.tile([P, TW], F32, tag="psy", name=f"psy_{it}_{h}")
                acc = mpool.tile([P, 2, TW], FP, tag="acc", name=f"acc_{it}_{h}")
                es_list = {}

                def emit_S(pr):
                    jt0 = 2 * pr
                    pss = ps_s.tile(
                        [P, 2, TW], F32, tag="pss", name=f"pss_{it}_{h}_{pr}"
                    )
                    for u in range(2):
                        nc.tensor.matmul(
                            pss[:, u, :],
                            lhsT=kT[:, (jt0 + u) * P:(jt0 + u + 1) * P],
                            rhs=qT[:, h, isl],
                            start=True,
                            stop=True,
                        )
                    es = espool.tile([P, 2, TW], FP, tag="es")
                    nc.scalar.activation(
                        out=es,
                        in_=pss,
                        func=mybir.ActivationFunctionType.Exp,
                        scale=SCALE,
                    )
                    if pr == 0:
                        nc.vector.tensor_copy(out=acc, in_=es)
                    else:
                        nc.vector.tensor_tensor(
                            out=acc, in0=acc, in1=es, op=mybir.AluOpType.add
                        )
                    es_list[pr] = es

                def emit_AV(pr):
                    jt0 = 2 * pr
                    es = es_list[pr]
                    for u in range(2):
                        nc.tensor.matmul(
                            psy,
                            lhsT=v_sb[:, jt0 + u, :],
                            rhs=es[:, u, :],
                            start=(jt0 + u == 0),
                            stop=False,
                        )

                # --- emission schedule: keep the PE 2 psum slots ahead ---
                emit_S(0)
                emit_S(1)
                psoA0 = out_proj_quarter(ic, 0)
                dias = None
                for p in range(noff):
                    emit_AV(p)
                    nxt = p + 2
                    if nxt < noff:
                        emit_S(nxt)
                    elif nxt == noff:
                        dias = emit_diag_S(it, h, acc)
                psoA1 = out_proj_quarter(ic, 1)
                emit_diag_AV(it, h, psy, dias[0], dias[1])
                psd = emit_den_yT(it, h, psy, acc)
                out_proj_copy(ic, 0, osb, psoA0)
                out_proj_copy(ic, 1, osb, psoA1)
                psoB2 = out_proj_quarter(ic, 2)
                out_proj_copy(ic, 2, osb, psoB2)
                psoB3 = out_proj_quarter(ic, 3)
                out_proj_copy(ic, 3, osb, psoB3)
                finish_yT(it, h, psy, psd)
        # Tail chunks: attention is done, so the S-pair pool (2 slots) is
        # also free and gives quarter-to-quarter pipelining.
        for h in range(NH):
            ic = (NT - 1) * (TW // P) + h
            osb = opool.tile([P, C], FP, tag="osb", name=f"osb_{ic}")
            for ot in range(4):
                pso = out_proj_quarter(ic, ot)
                out_proj_copy(ic, ot, osb, pso)

    nc.compile()
    return nc


def _causal_mask_tiles():
    # [128, 2, 128] tril (key_in_tile <= query_in_block), both slots equal.
    j = np.arange(P)[:, None, None]
    i = np.arange(P)[None, None, :]
    return np.broadcast_to(j <= i, (P, 2, P)).astype(np.float16)


def kernel(x, Wkv, bkv, Wq, bq, Wp, bp):
    global LAST_RESULT
    x = np.asarray(x, np.float32)
    Wkv = np.asarray(Wkv, np.float32)
    bkv = np.asarray(bkv, np.float32)
    Wq = np.asarray(Wq, np.float32)
    bq = np.asarray(bq, np.float32)
    Wp = np.asarray(Wp, np.float32)
    bp = np.asarray(bp, np.float32)

    if "nc" not in _CACHE:
        _CACHE["nc"] = _build_bass()
    nc = _CACHE["nc"]

    mask = _causal_mask_tiles()
    CG = C // G  # 512 columns per kv head in the k/v halves of Wkv

    in_maps = []
    for b in range(B):
        # xt packed: [p, n, ko, tw] = x[b].T[ko*128+p, n*512+tw]
        xtb = x[b].T.astype(np.float16).reshape(KT, P, NT, TW)
        xt_packed = np.ascontiguousarray(xtb.transpose(1, 2, 0, 3))
        for g in range(HKV):
            heads = [g + HKV * u for u in range(NH)]  # h % HKV == g
            wq_g = np.concatenate(
                [Wq[:, h * HD:(h + 1) * HD] for h in heads], axis=1
            ).astype(np.float16)
            wq_p = np.ascontiguousarray(
                wq_g.reshape(KT, P, NH * HD).transpose(1, 0, 2)
            )
            bq_g = np.concatenate([bq[h * HD:(h + 1) * HD] for h in heads])
            bq_p = np.ascontiguousarray(
                bq_g.reshape(NH, P).T.astype(np.float32)
            )
            wp_g = np.concatenate(
                [Wp[h * HD:(h + 1) * HD, :] for h in heads], axis=0
            ).astype(np.float16)
            wp_p = np.ascontiguousarray(wp_g.reshape(NH, P, C).transpose(1, 0, 2))
            wk_g = Wkv[:, g * HD:(g + 1) * HD].astype(np.float16)
            wk_p = np.ascontiguousarray(wk_g.reshape(KT, P, HD).transpose(1, 0, 2))
            wv_g = Wkv[:, CG + g * HD:CG + (g + 1) * HD].astype(np.float16)
            wv_p = np.ascontiguousarray(wv_g.reshape(KT, P, HD).transpose(1, 0, 2))
            bk_g = np.ascontiguousarray(
                bkv[g * HD:(g + 1) * HD].reshape(P, 1), np.float32
            )
            bv_g = np.ascontiguousarray(
                bkv[CG + g * HD:CG + (g + 1) * HD], np.float32
            )
            in_maps.append(
                {
                    "ident": np.eye(P, dtype=np.float16),
                    "xt": xt_packed,
                    "wq": wq_p,
                    "wk": wk_p,
                    "wv": wv_p,
                    "wp": wp_p,
                    "bq": bq_p,
                    "bk": bk_g,
                    "bv": bv_g,
                    "mask": mask,
                }
            )

    res = bass_utils.run_bass_kernel_spmd(nc, in_maps, core_ids=list(range(B * HKV)))
    LAST_RESULT = res

    out = np.zeros((B, T, C), np.float32)
    for b in range(B):
        acc = np.zeros((T, C), np.float32)
        for g in range(HKV):
            acc += res.results[b * HKV + g]["out"]
        out[b] = acc + bp[None, :]
    return out
